# revision 1
# baseline (speedup 1.0000x reference)
"""Trainium2 Bass kernel for DetectionSegmentationConsistency loss.

Per-box sums over seg-mask rectangles are computed as a masked matmul:
  diff  = footpath - driveway                    (DVE, bf16 out)
  T     = R^T.T @ diff  accumulated over 8 row-chunks in PSUM
          where R^T[h, n] = (y1[n] <= h < y2[n]) row-range indicator (bf16)
  S[n]  = sum_x T[n, x] * (x1[n] <= x < x2[n])   (two fused scalar_tensor_tensor)
  loss += relu(S[n]) * conf[n] * valid[n] / area[n]

Data-parallel across 8 NeuronCores: each core takes 4 of the 32 batch images
(only seg classes 1 and 2 are shipped) and emits one partial-sum scalar;
host adds the 8 partials and divides by B*N.

Instruction-dependency hygiene: walrus allows very few semaphore waits per
instruction, so every tile is written by at most one DMA, and cross-engine
fan-in is kept minimal (e.g. iotas are bounced through a DVE copy).
"""
import numpy as np
from contextlib import ExitStack

import concourse.bass as bass
import concourse.bacc as bacc
import concourse.tile as tile
from concourse import mybir
from concourse.bass_utils import run_bass_kernel_spmd

F32 = mybir.dt.float32
BF16 = mybir.dt.bfloat16
I32 = mybir.dt.int32

B, N, H, W = 32, 300, 1024, 1024
NCORES = 8
BC = B // NCORES                # images per core
KCH = H // 128                  # 8 row chunks
NG = [128, 128, 44]             # box groups along partitions
GS = [0, 128, 256]
CONF_THRESH = 0.3
MAGIC = 12582912.0              # 1.5 * 2^23: fp32 round-to-nearest-int trick

AluOp = mybir.AluOpType
Act = mybir.ActivationFunctionType


def _floor_clip(nc, pool, val_ap, out_ap, p, lo, hi):
    """out = clip(floor(val), lo, hi), exact fp32 (magic-number RN + fixup)."""
    fd = val_ap.shape[1]
    r = pool.tile([128, fd], F32, tag="fc_r", name="fc_r")
    gt = pool.tile([128, fd], F32, tag="fc_g", name="fc_g")
    nc.vector.tensor_scalar(
        out=r[:p], in0=val_ap, scalar1=MAGIC, scalar2=MAGIC,
        op0=AluOp.add, op1=AluOp.subtract)
    nc.vector.tensor_tensor(out=gt[:p], in0=r[:p], in1=val_ap, op=AluOp.is_gt)
    nc.vector.tensor_tensor(out=r[:p], in0=r[:p], in1=gt[:p], op=AluOp.subtract)
    nc.vector.tensor_scalar(
        out=out_ap, in0=r[:p], scalar1=float(lo), scalar2=float(hi),
        op0=AluOp.max, op1=AluOp.min)


def build_bass():
    # Bacc (not raw Bass): its finalize() runs move_matmul_waits_to_ldweights
    # + generate_event_semaphores, which legalize multi-sem waits for walrus.
    nc = bacc.Bacc()
    seg = nc.declare_dram_parameter("seg2", [BC, 2, H, W], F32, isOutput=False)
    boxes = nc.declare_dram_parameter("boxes", [BC, N, 4], F32, isOutput=False)
    conf = nc.declare_dram_parameter("conf", [BC, N], F32, isOutput=False)
    out = nc.declare_dram_parameter("out", [1, 1], F32, isOutput=True)

    with tile.TileContext(nc) as tc, ExitStack() as ctx:
        consts = ctx.enter_context(tc.tile_pool(name="consts", bufs=1))
        boxp = ctx.enter_context(tc.tile_pool(name="boxp", bufs=1))
        scratch = ctx.enter_context(tc.tile_pool(name="scratch", bufs=4))
        segp = ctx.enter_context(tc.tile_pool(name="segp", bufs=3))
        diffp = ctx.enter_context(tc.tile_pool(name="diffp", bufs=4))
        maskp = ctx.enter_context(tc.tile_pool(name="maskp", bufs=4))
        bcp = ctx.enter_context(tc.tile_pool(name="bcp", bufs=2))
        cmp_ = ctx.enter_context(tc.tile_pool(name="cmp", bufs=2))
        psum = ctx.enter_context(tc.tile_pool(name="psum", bufs=1, space="PSUM"))
        dramp = ctx.enter_context(tc.tile_pool(name="dramp", bufs=1, space="DRAM"))

        # ---- constants (iotas bounce through DVE so consumers only dep DVE) --
        iotaF_i = consts.tile([128, W], I32)
        nc.gpsimd.iota(iotaF_i, pattern=[[1, W]], base=0, channel_multiplier=0)
        iotaF = consts.tile([128, W], F32)
        nc.vector.tensor_copy(iotaF, iotaF_i)

        iotaP_i = consts.tile([128, KCH], I32)
        nc.gpsimd.iota(iotaP_i, pattern=[[128, KCH]], base=0, channel_multiplier=1)
        iotaP = consts.tile([128, KCH], F32)  # [p, k] = p + 128*k
        nc.vector.tensor_copy(iotaP, iotaP_i)

        ones_col = consts.tile([128, 1], F32)
        nc.vector.memset(ones_col, 1.0)

        # ---- per-box params, column layout (boxes on partitions) ----
        x1c, x2c, wgt, scol = [], [], [], []
        for g in range(3):
            p, s = NG[g], GS[g]
            bx = boxp.tile([128, BC, 4], F32, tag=f"bx{g}")
            nc.sync.dma_start(
                out=bx[:p], in_=boxes[:, s:s + p, :].rearrange("b n c -> n b c"))
            cf = boxp.tile([128, BC], F32, tag=f"cf{g}")
            nc.sync.dma_start(
                out=cf[:p], in_=conf[:, s:s + p].rearrange("b n -> n b"))

            cx, cy = bx[:p, :, 0], bx[:p, :, 1]
            ww, hh = bx[:p, :, 2], bx[:p, :, 3]
            w512 = scratch.tile([128, BC], F32, tag="w512")
            h512 = scratch.tile([128, BC], F32, tag="h512")
            nc.vector.tensor_scalar_mul(w512[:p], ww, 512.0)
            nc.vector.tensor_scalar_mul(h512[:p], hh, 512.0)

            x1g = boxp.tile([128, BC], F32, tag=f"x1c{g}")
            x2g = boxp.tile([128, BC], F32, tag=f"x2c{g}")
            y1g = boxp.tile([128, BC], F32, tag=f"y1c{g}")
            y2g = boxp.tile([128, BC], F32, tag=f"y2c{g}")
            for (vout, base, half, op1) in (
                (x1g, cx, w512, AluOp.subtract),
                (x2g, cx, w512, AluOp.add),
                (y1g, cy, h512, AluOp.subtract),
                (y2g, cy, h512, AluOp.add),
            ):
                vf = scratch.tile([128, BC], F32, tag="vf", name="vf")
                nc.vector.scalar_tensor_tensor(
                    out=vf[:p], in0=base, scalar=1024.0, in1=half[:p],
                    op0=AluOp.mult, op1=op1)
                _floor_clip(nc, scratch, vf[:p], vout[:p], p, 0.0, 1023.0)

            # weight = conf * (conf >= .3) / max(area, 1)
            dx = scratch.tile([128, BC], F32, tag="dx")
            dy = scratch.tile([128, BC], F32, tag="dy")
            nc.vector.tensor_tensor(out=dx[:p], in0=x2g[:p], in1=x1g[:p], op=AluOp.subtract)
            nc.vector.tensor_tensor(out=dy[:p], in0=y2g[:p], in1=y1g[:p], op=AluOp.subtract)
            area = scratch.tile([128, BC], F32, tag="area")
            nc.vector.tensor_tensor(out=area[:p], in0=dx[:p], in1=dy[:p], op=AluOp.mult)
            nc.vector.tensor_scalar_max(area[:p], area[:p], 1.0)
            rsa = scratch.tile([128, BC], F32, tag="rsa")
            nc.vector.reciprocal(rsa[:p], area[:p])
            vmask = scratch.tile([128, BC], F32, tag="vmask")
            nc.vector.tensor_scalar(
                out=vmask[:p], in0=cf[:p], scalar1=CONF_THRESH, scalar2=None,
                op0=AluOp.is_ge)
            wg = boxp.tile([128, BC], F32, tag=f"wgt{g}")
            nc.vector.tensor_tensor(out=wg[:p], in0=cf[:p], in1=vmask[:p], op=AluOp.mult)
            nc.vector.tensor_tensor(out=wg[:p], in0=wg[:p], in1=rsa[:p], op=AluOp.mult)

            sc = boxp.tile([128, BC], F32, tag=f"scol{g}")
            x1c.append(x1g); x2c.append(x2g)
            wgt.append(wg); scol.append(sc)

        # ---- y rows (images on partitions 0..3), all on DVE ----
        boxrow = boxp.tile([BC, N, 4], F32, tag="boxrow")
        nc.sync.dma_start(out=boxrow, in_=boxes[:, :, :])
        cyr, hhr = boxrow[:, :, 1], boxrow[:, :, 3]
        h512r = scratch.tile([BC, N], F32, tag="h512r")
        nc.vector.tensor_scalar_mul(h512r, hhr, 512.0)
        y1row = boxp.tile([BC, N], F32, tag="y1row")
        y2row = boxp.tile([BC, N], F32, tag="y2row")
        for (vout, op1) in ((y1row, AluOp.subtract), (y2row, AluOp.add)):
            vf = scratch.tile([BC, N], F32, tag="vfr", name="vfr")
            nc.vector.scalar_tensor_tensor(
                out=vf, in0=cyr, scalar=1024.0, in1=h512r,
                op0=AluOp.mult, op1=op1)
            _floor_clip(nc, scratch, vf[:BC], vout[:BC], BC, 0.0, 1023.0)

        # broadcast each image's y-row down 128 partitions via a DRAM bounce
        # (SBUF APs cannot have partition step 0, DRAM APs can)
        ybounce = dramp.tile([2, BC, N], F32)
        nc.gpsimd.dma_start(out=ybounce[0], in_=y1row[:, :])
        nc.gpsimd.dma_start(out=ybounce[1], in_=y2row[:, :])
        y1bc, y2bc = [], []
        for b in range(BC):
            for j in range(2):
                bc_sb = bcp.tile([128, N], F32, tag=f"ybc{j}", name=f"ybc{j}_{b}")
                nc.gpsimd.dma_start(
                    out=bc_sb, in_=ybounce[j, b:b + 1, :].to_broadcast((128, N)))
                (y1bc if j == 0 else y2bc).append(bc_sb)

        # ---- main loop over images ----
        for b in range(BC):
            tps = [psum.tile([NG[g], 1024], F32, tag=f"T{g}", name=f"T{g}_{b}")
                   for g in range(3)]
            for k in range(KCH):
                seg_t = segp.tile([128, 2, W], F32, tag="seg")
                nc.gpsimd.dma_start(
                    out=seg_t,
                    in_=seg[b, :, k * 128:(k + 1) * 128, :].rearrange("c p w -> p c w"))
                diff = diffp.tile([128, W], BF16, tag="diff")
                nc.vector.tensor_tensor(
                    out=diff, in0=seg_t[:, 1, :], in1=seg_t[:, 0, :], op=AluOp.subtract)

                m2 = maskp.tile([128, N], F32, tag="m2")
                nc.vector.tensor_scalar(
                    out=m2, in0=y2bc[b], scalar1=iotaP[:, k:k + 1], scalar2=None,
                    op0=AluOp.is_gt)
                rt = maskp.tile([128, N], BF16, tag="rt")
                nc.vector.scalar_tensor_tensor(
                    out=rt, in0=y1bc[b], scalar=iotaP[:, k:k + 1], in1=m2,
                    op0=AluOp.is_le, op1=AluOp.mult)

                for g in range(3):
                    p, s = NG[g], GS[g]
                    for half in range(2):
                        nc.tensor.matmul(
                            out=tps[g][:, half * 512:(half + 1) * 512],
                            lhsT=rt[:, s:s + p],
                            rhs=diff[:, half * 512:(half + 1) * 512],
                            start=(k == 0), stop=(k == KCH - 1))

            for g in range(3):
                p = NG[g]
                masked = cmp_.tile([NG[g], 1024], F32, tag="masked", name=f"masked{g}_{b}")
                nc.vector.scalar_tensor_tensor(
                    out=masked, in0=iotaF[:p], scalar=x1c[g][:p, b:b + 1],
                    in1=tps[g], op0=AluOp.is_ge, op1=AluOp.mult)
                junk = cmp_.tile([NG[g], 1024], F32, tag="junk", name=f"junk{g}_{b}")
                nc.vector.scalar_tensor_tensor(
                    out=junk, in0=iotaF[:p], scalar=x2c[g][:p, b:b + 1],
                    in1=masked, op0=AluOp.is_lt, op1=AluOp.mult,
                    accum_out=scol[g][:p, b:b + 1])

        # ---- final: relu(S)*wgt, reduce boxes+images, partition-reduce ----
        fin = psum.tile([1, 1], F32, tag="fin")
        for g in range(3):
            p = NG[g]
            pb = scratch.tile([128, BC], F32, tag="pb")
            nc.vector.tensor_tensor(out=pb[:p], in0=scol[g][:p], in1=wgt[g][:p], op=AluOp.mult)
            rl = scratch.tile([128, BC], F32, tag="rl")
            nc.vector.tensor_relu(rl[:p], pb[:p])
            rs = scratch.tile([128, 1], F32, tag="rs")
            nc.vector.reduce_sum(out=rs[:p], in_=rl[:p], axis=mybir.AxisListType.X)
            nc.tensor.matmul(
                out=fin, lhsT=ones_col[:p], rhs=rs[:p],
                start=(g == 0), stop=(g == 2))
        fsb = scratch.tile([1, 1], F32, tag="fsb")
        nc.scalar.copy(out=fsb, in_=fin)
        nc.sync.dma_start(out=out[0:1, 0:1], in_=fsb)

    nc.finalize()
    return nc


_NC_CACHE = None


def _get_nc():
    global _NC_CACHE
    if _NC_CACHE is None:
        _NC_CACHE = build_bass()
    return _NC_CACHE


def kernel(det_boxes, det_confidence, seg_masks):
    det_boxes = np.ascontiguousarray(np.asarray(det_boxes, dtype=np.float32))
    det_confidence = np.ascontiguousarray(np.asarray(det_confidence, dtype=np.float32))
    seg_masks = np.asarray(seg_masks, dtype=np.float32)

    nc = _get_nc()
    in_maps = []
    for i in range(NCORES):
        sl = slice(BC * i, BC * (i + 1))
        in_maps.append({
            "seg2": np.ascontiguousarray(seg_masks[sl, 1:3]),
            "boxes": det_boxes[sl],
            "conf": det_confidence[sl],
        })
    res = run_bass_kernel_spmd(nc, in_maps, list(range(NCORES)))
    parts = np.array([res.results[i]["out"][0, 0] for i in range(NCORES)],
                     dtype=np.float32)
    total = np.sum(parts, dtype=np.float32) / np.float32(B * N)
    return np.array(total, dtype=np.float32)



# revision 7
# speedup vs baseline: 1.3237x; 1.3237x over previous
"""Trainium2 Bass kernel for DetectionSegmentationConsistency loss (v2).

Algorithm (per image): box sums over seg-mask rectangles via masked matmul
  diff[y, x] = footpath - driveway                  (fp16)
  T[n, x]    = sum_y rt[y, n] * diff[y, x]          (PE, PSUM f32, 8 chunks)
  S[n]       = sum_x T[n, x] * (x1[n] <= x < x2[n]) (ACT copy + 2 DVE STT)
  loss      += relu(S[n]) * conf[n] * valid[n] / area[n]

v2 changes vs v1 (159 us):
  - seg classes shipped from host as fp16 (halves HBM traffic; rel err of a
    box sum from fp16 quantization is ~1e-3, far under the 2e-2 gate)
  - one 2 MB HWDGE DMA per image-half instead of 8x 1MB SWDGE DMAs; row
    mapping y = 512h + 4p + k keeps every descriptor 8 KB contiguous
  - boxes padded to 384 on host -> 3 uniform groups of 128, one DMA for all
    box params in column layout, box math vectorized over (group, image)
  - all index compares in fp16 (values <= 1023 are exact) -> DVE 2x/4x modes
  - PSUM drain split: ACT copies PSUM->fp16 SBUF, DVE does the two x-mask
    STTs at 4x rate with accum_out
Data-parallel: 4 of 32 images per core; host sums 8 partial scalars.
"""
import numpy as np
from contextlib import ExitStack

import concourse.bass as bass
import concourse.bacc as bacc
import concourse.tile as tile
from concourse import mybir
from concourse.bass_utils import run_bass_kernel_spmd

F32 = mybir.dt.float32
F16 = mybir.dt.float16
I32 = mybir.dt.int32

B, N, C, H, W = 32, 300, 3, 1024, 1024
NP = 384                        # padded box count (3 groups of 128)
NCORES = 8
BC = B // NCORES                # images per core
KCH = 8                         # row chunks (y = 512h + 4p + k', k = 4h + k')
CONF_THRESH = 0.3
MAGIC = 12582912.0              # 1.5 * 2^23: fp32 round-to-nearest-int trick

AluOp = mybir.AluOpType


def _floor_clip(nc, pool, val_ap, out_ap, p, lo, hi):
    """out = clip(floor(val), lo, hi); out may be fp16 (values are ints <=1023)."""
    fd = int(np.prod(val_ap.shape[1:]))
    r = pool.tile([128, fd], F32, tag="fc_r", name="fc_r")
    gt = pool.tile([128, fd], F32, tag="fc_g", name="fc_g")
    nc.vector.tensor_scalar(
        out=r[:p], in0=val_ap, scalar1=MAGIC, scalar2=MAGIC,
        op0=AluOp.add, op1=AluOp.subtract)
    nc.vector.tensor_tensor(out=gt[:p], in0=r[:p], in1=val_ap, op=AluOp.is_gt)
    nc.vector.tensor_tensor(out=r[:p], in0=r[:p], in1=gt[:p], op=AluOp.subtract)
    nc.vector.tensor_scalar(
        out=out_ap, in0=r[:p], scalar1=float(lo), scalar2=float(hi),
        op0=AluOp.max, op1=AluOp.min)


def build_bass():
    nc = bacc.Bacc()
    seg = nc.declare_dram_parameter("seg2", [BC, 2, H, W], F16, isOutput=False)
    boxc = nc.declare_dram_parameter("boxc", [128, 3, BC, 4], F32, isOutput=False)
    confc = nc.declare_dram_parameter("confc", [128, 3, BC], F32, isOutput=False)
    boxr = nc.declare_dram_parameter("boxr", [BC, NP, 4], F32, isOutput=False)
    out = nc.declare_dram_parameter("out", [1, 1], F32, isOutput=True)

    with tile.TileContext(nc) as tc, ExitStack() as ctx:
        consts = ctx.enter_context(tc.tile_pool(name="consts", bufs=1))
        boxp = ctx.enter_context(tc.tile_pool(name="boxp", bufs=1))
        scratch = ctx.enter_context(tc.tile_pool(name="scratch", bufs=4))
        segp = ctx.enter_context(tc.tile_pool(name="segp", bufs=3))
        diffp = ctx.enter_context(tc.tile_pool(name="diffp", bufs=3))
        maskp = ctx.enter_context(tc.tile_pool(name="maskp", bufs=2))
        drainp = ctx.enter_context(tc.tile_pool(name="drainp", bufs=3))
        psum = ctx.enter_context(tc.tile_pool(name="psum", bufs=3, space="PSUM"))
        psumf = ctx.enter_context(tc.tile_pool(name="psumf", bufs=1, space="PSUM"))
        dramp = ctx.enter_context(tc.tile_pool(name="dramp", bufs=1, space="DRAM"))

        # ---- constants ----
        # iotaP[p, k] = y-row of (partition p, chunk k) = 512*(k//4) + 4p + k%4
        iotaP_i = consts.tile([128, KCH], I32)
        nc.gpsimd.iota(iotaP_i[:, 0:4], pattern=[[1, 4]], base=0, channel_multiplier=4)
        nc.gpsimd.iota(iotaP_i[:, 4:8], pattern=[[1, 4]], base=512, channel_multiplier=4)
        iotaP = consts.tile([128, KCH], F32)
        nc.vector.tensor_copy(iotaP, iotaP_i)

        iotaF_i = consts.tile([128, W], I32)
        nc.gpsimd.iota(iotaF_i, pattern=[[1, W]], base=0, channel_multiplier=0)
        iotaF = consts.tile([128, W], F16)   # x coords 0..1023, exact in fp16
        nc.vector.tensor_copy(iotaF, iotaF_i)

        ones_col = consts.tile([128, 1], F32)
        nc.vector.memset(ones_col, 1.0)

        # ---- box params, column layout [n(part), group, image] ----
        bx = boxp.tile([128, 3, BC, 4], F32)
        nc.sync.dma_start(out=bx, in_=boxc[:, :, :, :])
        cf = boxp.tile([128, 3, BC], F32)
        nc.sync.dma_start(out=cf, in_=confc[:, :, :])

        cx, w_ = bx[:, :, :, 0], bx[:, :, :, 2]
        w512 = scratch.tile([128, 3, BC], F32, tag="w512")
        nc.vector.tensor_scalar_mul(w512, w_, 512.0)
        x1c = boxp.tile([128, 3, BC], F32)
        x2c = boxp.tile([128, 3, BC], F32)
        for (vout, op1) in ((x1c, AluOp.subtract), (x2c, AluOp.add)):
            vf = scratch.tile([128, 3, BC], F32, tag="vf", name=f"vf{op1}")
            nc.vector.scalar_tensor_tensor(
                out=vf, in0=cx, scalar=1024.0, in1=w512,
                op0=AluOp.mult, op1=op1)
            _floor_clip(nc, scratch, vf[:128], vout[:128], 128, 0.0, 1023.0)
        # y bounds in column layout (only for area/validity weight)
        cy, h_ = bx[:, :, :, 1], bx[:, :, :, 3]
        h512 = scratch.tile([128, 3, BC], F32, tag="h512")
        nc.vector.tensor_scalar_mul(h512, h_, 512.0)
        y1c = scratch.tile([128, 3, BC], F32, tag="y1c")
        y2c = scratch.tile([128, 3, BC], F32, tag="y2c")
        for (vout, op1) in ((y1c, AluOp.subtract), (y2c, AluOp.add)):
            vf = scratch.tile([128, 3, BC], F32, tag="vf", name=f"vfy{op1}")
            nc.vector.scalar_tensor_tensor(
                out=vf, in0=cy, scalar=1024.0, in1=h512,
                op0=AluOp.mult, op1=op1)
            _floor_clip(nc, scratch, vf[:128], vout[:128], 128, 0.0, 1023.0)

        # weight = conf * (conf >= .3) / max(area, 1)
        dx = scratch.tile([128, 3, BC], F32, tag="dx")
        dy = scratch.tile([128, 3, BC], F32, tag="dy")
        nc.vector.tensor_tensor(out=dx, in0=x2c, in1=x1c, op=AluOp.subtract)
        nc.vector.tensor_tensor(out=dy, in0=y2c, in1=y1c, op=AluOp.subtract)
        area = scratch.tile([128, 3, BC], F32, tag="area")
        nc.vector.tensor_tensor(out=area, in0=dx, in1=dy, op=AluOp.mult)
        nc.vector.tensor_scalar_max(area, area, 1.0)
        rsa = scratch.tile([128, 3, BC], F32, tag="rsa")
        nc.vector.reciprocal(rsa, area)
        vmask = scratch.tile([128, 3, BC], F32, tag="vmask")
        nc.vector.tensor_scalar(
            out=vmask, in0=cf, scalar1=CONF_THRESH, scalar2=None, op0=AluOp.is_ge)
        wgt = boxp.tile([128, 3, BC], F32)
        nc.vector.tensor_tensor(out=wgt, in0=cf, in1=vmask, op=AluOp.mult)
        nc.vector.tensor_tensor(out=wgt, in0=wgt, in1=rsa, op=AluOp.mult)

        # ---- y rows (row layout), fp16 out, bounce to DRAM, broadcast ----
        boxrow = boxp.tile([BC, NP, 4], F32)
        nc.sync.dma_start(out=boxrow, in_=boxr[:, :, :])
        cyr, hhr = boxrow[:, :, 1], boxrow[:, :, 3]
        h512r = scratch.tile([BC, NP], F32, tag="h512r")
        nc.vector.tensor_scalar_mul(h512r, hhr, 512.0)
        y16 = boxp.tile([BC, 2, NP], F16)      # [image, {y1,y2}, n]
        for j, op1 in ((0, AluOp.subtract), (1, AluOp.add)):
            vf = scratch.tile([BC, NP], F32, tag="vfr", name=f"vfr{j}")
            nc.vector.scalar_tensor_tensor(
                out=vf, in0=cyr, scalar=1024.0, in1=h512r,
                op0=AluOp.mult, op1=op1)
            _floor_clip(nc, scratch, vf[:BC], y16[:BC, j, :], BC, 0.0, 1023.0)

        ybounce = dramp.tile([2, BC, NP], F16)
        nc.sync.dma_start(out=ybounce.rearrange("j b n -> b j n"),
                          in_=y16[:, :, :])
        # ybc[p, j, (b, n)] = y_j bound, same on all 128 partitions
        ybc = boxp.tile([128, 2, BC * NP], F16)
        nc.gpsimd.dma_start(
            out=ybc.rearrange("p j n -> p (j n)"),
            in_=ybounce.rearrange("j b n -> (j b n)")[None, :].to_broadcast(
                (128, 2 * BC * NP)))

        # ---- row masks rt[p, k, (b, n)] = (y1 <= y(p,k) < y2), fp16 ----
        rt = boxp.tile([128, KCH, BC * NP], F16)
        for k in range(KCH):
            m2 = maskp.tile([128, BC * NP], F16, tag="m2")
            nc.vector.tensor_scalar(
                out=m2, in0=ybc[:, 1, :], scalar1=iotaP[:, k:k + 1], scalar2=None,
                op0=AluOp.is_gt)
            nc.vector.scalar_tensor_tensor(
                out=rt[:, k, :], in0=ybc[:, 0, :], scalar=iotaP[:, k:k + 1],
                in1=m2, op0=AluOp.is_le, op1=AluOp.mult)

        # ---- per-box column sums S -> scol[n, g, b] ----
        scol = boxp.tile([128, 3, BC], F32)

        for b in range(BC):
            tps = [psum.tile([128, 1024], F32, tag="T", name=f"T{g}_{b}")
                   for g in range(3)]
            for h in range(2):
                seg_t = segp.tile([128, 2, 4, W], F16, tag="seg")
                nc.sync.dma_start(
                    out=seg_t,
                    in_=seg[b, :, 512 * h:512 * (h + 1), :]
                    .rearrange("c (p k) w -> p c k w", k=4))
                diff = diffp.tile([128, 4, W], F16, tag="diff")
                nc.vector.scalar_tensor_tensor(
                    out=diff, in0=seg_t[:, 1, :, :], scalar=1.0,
                    in1=seg_t[:, 0, :, :], op0=AluOp.mult, op1=AluOp.subtract)
                for kk in range(4):
                    k = 4 * h + kk
                    for g in range(3):
                        lo = b * NP + g * 128
                        for half in range(2):
                            nc.tensor.matmul(
                                out=tps[g][:, half * 512:(half + 1) * 512],
                                lhsT=rt[:, k, lo:lo + 128],
                                rhs=diff[:, kk, half * 512:(half + 1) * 512],
                                start=(k == 0), stop=(k == KCH - 1))

            for g in range(3):
                t16 = drainp.tile([128, 1024], F16, tag="t16", name=f"t16_{g}_{b}")
                nc.scalar.copy(out=t16, in_=tps[g])
                q1 = drainp.tile([128, 1024], F16, tag="q1", name=f"q1_{g}_{b}")
                nc.vector.scalar_tensor_tensor(
                    out=q1, in0=iotaF, scalar=x1c[:, g, b:b + 1], in1=t16,
                    op0=AluOp.is_ge, op1=AluOp.mult)
                q2 = drainp.tile([128, 1024], F16, tag="q2", name=f"q2_{g}_{b}")
                nc.vector.scalar_tensor_tensor(
                    out=q2, in0=iotaF, scalar=x2c[:, g, b:b + 1], in1=q1,
                    op0=AluOp.is_lt, op1=AluOp.mult,
                    accum_out=scol[:, g, b:b + 1])

        # ---- final: relu(S)*wgt, reduce over (g, b) and partitions ----
        pb = scratch.tile([128, 3, BC], F32, tag="pb")
        nc.vector.tensor_tensor(out=pb, in0=scol, in1=wgt, op=AluOp.mult)
        rl = scratch.tile([128, 3, BC], F32, tag="rl")
        nc.vector.tensor_relu(rl, pb)
        rs = scratch.tile([128, 1], F32, tag="rs")
        nc.vector.reduce_sum(out=rs, in_=rl.rearrange("p a b -> p (a b)"),
                             axis=mybir.AxisListType.X)
        fin = psumf.tile([1, 1], F32, tag="fin")
        nc.tensor.matmul(out=fin, lhsT=ones_col, rhs=rs, start=True, stop=True)
        fsb = scratch.tile([1, 1], F32, tag="fsb")
        nc.scalar.copy(out=fsb, in_=fin)
        nc.sync.dma_start(out=out[0:1, 0:1], in_=fsb)

    nc.finalize()
    return nc


_NC_CACHE = None


def _get_nc():
    global _NC_CACHE
    if _NC_CACHE is None:
        _NC_CACHE = build_bass()
    return _NC_CACHE


def make_in_maps(det_boxes, det_confidence, seg_masks):
    det_boxes = np.asarray(det_boxes, dtype=np.float32)
    det_confidence = np.asarray(det_confidence, dtype=np.float32)
    seg16 = np.asarray(seg_masks)[:, 1:3].astype(np.float16)

    boxes_pad = np.zeros((B, NP, 4), dtype=np.float32)
    boxes_pad[:, :N] = det_boxes
    conf_pad = np.zeros((B, NP), dtype=np.float32)
    conf_pad[:, :N] = det_confidence

    in_maps = []
    for i in range(NCORES):
        sl = slice(BC * i, BC * (i + 1))
        bp = boxes_pad[sl]                                   # [BC, 384, 4]
        boxc = np.ascontiguousarray(
            bp.reshape(BC, 3, 128, 4).transpose(2, 1, 0, 3))  # [128, 3, BC, 4]
        confc = np.ascontiguousarray(
            conf_pad[sl].reshape(BC, 3, 128).transpose(2, 1, 0))  # [128, 3, BC]
        in_maps.append({
            "seg2": np.ascontiguousarray(seg16[sl]),
            "boxc": boxc,
            "confc": confc,
            "boxr": np.ascontiguousarray(bp),
        })
    return in_maps


def combine_outputs(results):
    parts = np.array([results[i]["out"][0, 0] for i in range(NCORES)],
                     dtype=np.float32)
    return np.array(np.sum(parts, dtype=np.float32) / np.float32(B * N),
                    dtype=np.float32)


def kernel(det_boxes, det_confidence, seg_masks):
    nc = _get_nc()
    in_maps = make_in_maps(det_boxes, det_confidence, seg_masks)
    res = run_bass_kernel_spmd(nc, in_maps, list(range(NCORES)))
    return combine_outputs(res.results)


# revision 18
# speedup vs baseline: 1.5125x; 1.1426x over previous
"""Trainium2 Bass kernel for DetectionSegmentationConsistency loss (v2).

Algorithm (per image): box sums over seg-mask rectangles via masked matmul
  diff[y, x] = footpath - driveway                  (fp16)
  T[n, x]    = sum_y rt[y, n] * diff[y, x]          (PE, PSUM f32, 8 chunks)
  S[n]       = sum_x T[n, x] * (x1[n] <= x < x2[n]) (ACT copy + 2 DVE STT)
  loss      += relu(S[n]) * conf[n] * valid[n] / area[n]

v2 changes vs v1 (159 us):
  - seg classes shipped from host as fp16 (halves HBM traffic; rel err of a
    box sum from fp16 quantization is ~1e-3, far under the 2e-2 gate)
  - one 2 MB HWDGE DMA per image-half instead of 8x 1MB SWDGE DMAs; row
    mapping y = 512h + 4p + k keeps every descriptor 8 KB contiguous
  - boxes padded to 384 on host -> 3 uniform groups of 128, one DMA for all
    box params in column layout, box math vectorized over (group, image)
  - all index compares in fp16 (values <= 1023 are exact) -> DVE 2x/4x modes
  - PSUM drain split: ACT copies PSUM->fp16 SBUF, DVE does the two x-mask
    STTs at 4x rate with accum_out
Data-parallel: 4 of 32 images per core; host sums 8 partial scalars.
"""
import numpy as np
from contextlib import ExitStack

import concourse.bass as bass
import concourse.bacc as bacc
import concourse.tile as tile
from concourse import mybir
from concourse.bass_utils import run_bass_kernel_spmd

F32 = mybir.dt.float32
F16 = mybir.dt.float16
I32 = mybir.dt.int32

B, N, C, H, W = 32, 300, 3, 1024, 1024
NP = 384                        # padded box count (3 groups of 128)
NCORES = 8
BC = B // NCORES                # images per core
KCH = 8                         # row chunks (y = 512h + 4p + k', k = 4h + k')
CONF_THRESH = 0.3
MAGIC = 12582912.0              # 1.5 * 2^23: fp32 round-to-nearest-int trick

AluOp = mybir.AluOpType


def _floor_clip(nc, pool, val_ap, out_ap, p, lo, hi):
    """out = clip(floor(val), lo, hi); out may be fp16 (values are ints <=1023)."""
    fd = int(np.prod(val_ap.shape[1:]))
    r = pool.tile([128, fd], F32, tag="fc_r", name="fc_r")
    gt = pool.tile([128, fd], F32, tag="fc_g", name="fc_g")
    nc.vector.tensor_scalar(
        out=r[:p], in0=val_ap, scalar1=MAGIC, scalar2=MAGIC,
        op0=AluOp.add, op1=AluOp.subtract)
    nc.vector.tensor_tensor(out=gt[:p], in0=r[:p], in1=val_ap, op=AluOp.is_gt)
    nc.vector.tensor_tensor(out=r[:p], in0=r[:p], in1=gt[:p], op=AluOp.subtract)
    nc.vector.tensor_scalar(
        out=out_ap, in0=r[:p], scalar1=float(lo), scalar2=float(hi),
        op0=AluOp.max, op1=AluOp.min)


def build_bass():
    nc = bacc.Bacc()
    seg = nc.declare_dram_parameter("seg2", [BC, 2, H, W], F16, isOutput=False)
    boxc = nc.declare_dram_parameter("boxc", [128, 3, BC, 4], F32, isOutput=False)
    confc = nc.declare_dram_parameter("confc", [128, 3, BC], F32, isOutput=False)
    boxr = nc.declare_dram_parameter("boxr", [BC, NP, 4], F32, isOutput=False)
    out = nc.declare_dram_parameter("out", [1, 1], F32, isOutput=True)

    with tile.TileContext(nc) as tc, ExitStack() as ctx:
        consts = ctx.enter_context(tc.tile_pool(name="consts", bufs=1))
        boxp = ctx.enter_context(tc.tile_pool(name="boxp", bufs=1))
        scratch = ctx.enter_context(tc.tile_pool(name="scratch", bufs=4))
        segp = ctx.enter_context(tc.tile_pool(name="segp", bufs=3))
        diffp = ctx.enter_context(tc.tile_pool(name="diffp", bufs=3))
        maskp = ctx.enter_context(tc.tile_pool(name="maskp", bufs=2))
        drainp = ctx.enter_context(tc.tile_pool(name="drainp", bufs=3))
        psum = ctx.enter_context(tc.tile_pool(name="psum", bufs=3, space="PSUM"))
        psumf = ctx.enter_context(tc.tile_pool(name="psumf", bufs=1, space="PSUM"))
        dramp = ctx.enter_context(tc.tile_pool(name="dramp", bufs=1, space="DRAM"))

        # ---- constants ----
        # iotaP[p, k] = y-row of (partition p, chunk k) = 512*(k//4) + 4p + k%4
        iotaP_i = consts.tile([128, KCH], I32)
        nc.gpsimd.iota(iotaP_i[:, 0:4], pattern=[[1, 4]], base=0, channel_multiplier=4)
        nc.gpsimd.iota(iotaP_i[:, 4:8], pattern=[[1, 4]], base=512, channel_multiplier=4)
        iotaP = consts.tile([128, KCH], F32)
        nc.vector.tensor_copy(iotaP, iotaP_i)

        iotaF_i = consts.tile([128, W], I32)
        nc.gpsimd.iota(iotaF_i, pattern=[[1, W]], base=0, channel_multiplier=0)
        iotaF = consts.tile([128, W], F16)   # x coords 0..1023, exact in fp16
        nc.vector.tensor_copy(iotaF, iotaF_i)

        ones_col = consts.tile([128, 1], F32)
        nc.vector.memset(ones_col, 1.0)

        # ---- box params, column layout [n(part), group, image] ----
        bx = boxp.tile([128, 3, BC, 4], F32)
        nc.sync.dma_start(out=bx, in_=boxc[:, :, :, :])
        cf = boxp.tile([128, 3, BC], F32)
        nc.sync.dma_start(out=cf, in_=confc[:, :, :])

        cx, w_ = bx[:, :, :, 0], bx[:, :, :, 2]
        w512 = scratch.tile([128, 3, BC], F32, tag="w512")
        nc.vector.tensor_scalar_mul(w512, w_, 512.0)
        x1c = boxp.tile([128, 3, BC], F32)
        x2c = boxp.tile([128, 3, BC], F32)
        for (vout, op1) in ((x1c, AluOp.subtract), (x2c, AluOp.add)):
            vf = scratch.tile([128, 3, BC], F32, tag="vf", name=f"vf{op1}")
            nc.vector.scalar_tensor_tensor(
                out=vf, in0=cx, scalar=1024.0, in1=w512,
                op0=AluOp.mult, op1=op1)
            _floor_clip(nc, scratch, vf[:128], vout[:128], 128, 0.0, 1023.0)
        # y bounds in column layout (only for area/validity weight)
        cy, h_ = bx[:, :, :, 1], bx[:, :, :, 3]
        h512 = scratch.tile([128, 3, BC], F32, tag="h512")
        nc.vector.tensor_scalar_mul(h512, h_, 512.0)
        y1c = scratch.tile([128, 3, BC], F32, tag="y1c")
        y2c = scratch.tile([128, 3, BC], F32, tag="y2c")
        for (vout, op1) in ((y1c, AluOp.subtract), (y2c, AluOp.add)):
            vf = scratch.tile([128, 3, BC], F32, tag="vf", name=f"vfy{op1}")
            nc.vector.scalar_tensor_tensor(
                out=vf, in0=cy, scalar=1024.0, in1=h512,
                op0=AluOp.mult, op1=op1)
            _floor_clip(nc, scratch, vf[:128], vout[:128], 128, 0.0, 1023.0)

        # weight = conf * (conf >= .3) / max(area, 1)
        dx = scratch.tile([128, 3, BC], F32, tag="dx")
        dy = scratch.tile([128, 3, BC], F32, tag="dy")
        nc.vector.tensor_tensor(out=dx, in0=x2c, in1=x1c, op=AluOp.subtract)
        nc.vector.tensor_tensor(out=dy, in0=y2c, in1=y1c, op=AluOp.subtract)
        area = scratch.tile([128, 3, BC], F32, tag="area")
        nc.vector.tensor_tensor(out=area, in0=dx, in1=dy, op=AluOp.mult)
        nc.vector.tensor_scalar_max(area, area, 1.0)
        rsa = scratch.tile([128, 3, BC], F32, tag="rsa")
        nc.vector.reciprocal(rsa, area)
        vmask = scratch.tile([128, 3, BC], F32, tag="vmask")
        nc.vector.tensor_scalar(
            out=vmask, in0=cf, scalar1=CONF_THRESH, scalar2=None, op0=AluOp.is_ge)
        wgt = boxp.tile([128, 3, BC], F32)
        nc.vector.tensor_tensor(out=wgt, in0=cf, in1=vmask, op=AluOp.mult)
        nc.vector.tensor_tensor(out=wgt, in0=wgt, in1=rsa, op=AluOp.mult)

        # ---- y rows (row layout), fp16 out, bounce to DRAM, broadcast ----
        boxrow = boxp.tile([BC, NP, 4], F32)
        nc.sync.dma_start(out=boxrow, in_=boxr[:, :, :])
        cyr, hhr = boxrow[:, :, 1], boxrow[:, :, 3]
        h512r = scratch.tile([BC, NP], F32, tag="h512r")
        nc.vector.tensor_scalar_mul(h512r, hhr, 512.0)
        y16 = boxp.tile([BC, 2, NP], F16)      # [image, {y1,y2}, n]
        for j, op1 in ((0, AluOp.subtract), (1, AluOp.add)):
            vf = scratch.tile([BC, NP], F32, tag="vfr", name=f"vfr{j}")
            nc.vector.scalar_tensor_tensor(
                out=vf, in0=cyr, scalar=1024.0, in1=h512r,
                op0=AluOp.mult, op1=op1)
            _floor_clip(nc, scratch, vf[:BC], y16[:BC, j, :], BC, 0.0, 1023.0)

        ybounce = dramp.tile([2, BC, NP], F16)
        nc.sync.dma_start(out=ybounce.rearrange("j b n -> b j n"),
                          in_=y16[:, :, :])
        # ybc[p, j, (b, n)] = y_j bound, same on all 128 partitions
        ybc = boxp.tile([128, 2, BC * NP], F16)
        nc.gpsimd.dma_start(
            out=ybc.rearrange("p j n -> p (j n)"),
            in_=ybounce.rearrange("j b n -> (j b n)")[None, :].to_broadcast(
                (128, 2 * BC * NP)))

        # ---- row masks rt[p, k, (b, n)] = (y1 <= y(p,k) < y2), fp16 ----
        # [y1<=v<y2] = [y1<=v] - [y2<=v]; ts ops hit the 4x DVE mode,
        # tensor_tensor hits 2x (STT runs at 1x on TRN2 -> avoid for bulk).
        rt = boxp.tile([128, KCH, BC * NP], F16)
        for k in range(KCH):
            ms1 = maskp.tile([128, BC * NP], F16, tag="ms1")
            ms2 = maskp.tile([128, BC * NP], F16, tag="ms2")
            nc.vector.tensor_scalar(
                out=ms1, in0=ybc[:, 0, :], scalar1=iotaP[:, k:k + 1], scalar2=None,
                op0=AluOp.is_le)
            nc.vector.tensor_scalar(
                out=ms2, in0=ybc[:, 1, :], scalar1=iotaP[:, k:k + 1], scalar2=None,
                op0=AluOp.is_le)
            nc.vector.tensor_tensor(
                out=rt[:, k, :], in0=ms1, in1=ms2, op=AluOp.subtract)

        # ---- per-box column sums S -> scol[n, g, b] ----
        scol = boxp.tile([128, 3, BC], F32)

        for b in range(BC):
            tps = [psum.tile([128, 1024], F32, tag="T", name=f"T{g}_{b}")
                   for g in range(3)]
            for h in range(2):
                # driveway plane is shipped sign-flipped, so diff = c1 + c0
                seg_t = segp.tile([128, 2, 4, W], F16, tag="seg")
                nc.sync.dma_start(
                    out=seg_t,
                    in_=seg[b, :, 512 * h:512 * (h + 1), :]
                    .rearrange("c (p k) w -> p c k w", k=4))
                diff = diffp.tile([128, 4, W], F16, tag="diff")
                nc.vector.tensor_tensor(
                    out=diff, in0=seg_t[:, 1, :, :], in1=seg_t[:, 0, :, :],
                    op=AluOp.add)
                for kk in range(4):
                    k = 4 * h + kk
                    for g in range(3):
                        lo = b * NP + g * 128
                        for half in range(2):
                            nc.tensor.matmul(
                                out=tps[g][:, half * 512:(half + 1) * 512],
                                lhsT=rt[:, k, lo:lo + 128],
                                rhs=diff[:, kk, half * 512:(half + 1) * 512],
                                start=(k == 0), stop=(k == KCH - 1))

            for g in range(3):
                t16 = drainp.tile([128, 1024], F16, tag="t16", name=f"t16_{g}_{b}")
                nc.scalar.copy(out=t16, in_=tps[g])
                q1 = drainp.tile([128, 1024], F16, tag="q1", name=f"q1_{g}_{b}")
                nc.vector.scalar_tensor_tensor(
                    out=q1, in0=iotaF, scalar=x1c[:, g, b:b + 1], in1=t16,
                    op0=AluOp.is_ge, op1=AluOp.mult)
                q2 = drainp.tile([128, 1024], F16, tag="q2", name=f"q2_{g}_{b}")
                nc.vector.scalar_tensor_tensor(
                    out=q2, in0=iotaF, scalar=x2c[:, g, b:b + 1], in1=q1,
                    op0=AluOp.is_lt, op1=AluOp.mult,
                    accum_out=scol[:, g, b:b + 1])

        # ---- final: relu(S)*wgt, reduce over (g, b) and partitions ----
        pb = scratch.tile([128, 3, BC], F32, tag="pb")
        nc.vector.tensor_tensor(out=pb, in0=scol, in1=wgt, op=AluOp.mult)
        rl = scratch.tile([128, 3, BC], F32, tag="rl")
        nc.vector.tensor_relu(rl, pb)
        rs = scratch.tile([128, 1], F32, tag="rs")
        nc.vector.reduce_sum(out=rs, in_=rl.rearrange("p a b -> p (a b)"),
                             axis=mybir.AxisListType.X)
        fin = psumf.tile([1, 1], F32, tag="fin")
        nc.tensor.matmul(out=fin, lhsT=ones_col, rhs=rs, start=True, stop=True)
        fsb = scratch.tile([1, 1], F32, tag="fsb")
        nc.scalar.copy(out=fsb, in_=fin)
        nc.sync.dma_start(out=out[0:1, 0:1], in_=fsb)

    nc.finalize()
    return nc


_NC_CACHE = None


def _get_nc():
    global _NC_CACHE
    if _NC_CACHE is None:
        _NC_CACHE = build_bass()
    return _NC_CACHE


def make_in_maps(det_boxes, det_confidence, seg_masks):
    det_boxes = np.asarray(det_boxes, dtype=np.float32)
    det_confidence = np.asarray(det_confidence, dtype=np.float32)
    seg16 = np.asarray(seg_masks)[:, 1:3].astype(np.float16)
    # ship the driveway plane with its sign bit flipped so the DMA CCE
    # accumulate (add-only) computes footpath - driveway on the fly
    seg16 = np.ascontiguousarray(seg16)
    u = seg16.view(np.uint16)
    u[:, 0] ^= np.uint16(0x8000)

    boxes_pad = np.zeros((B, NP, 4), dtype=np.float32)
    boxes_pad[:, :N] = det_boxes
    conf_pad = np.zeros((B, NP), dtype=np.float32)
    conf_pad[:, :N] = det_confidence

    in_maps = []
    for i in range(NCORES):
        sl = slice(BC * i, BC * (i + 1))
        bp = boxes_pad[sl]                                   # [BC, 384, 4]
        boxc = np.ascontiguousarray(
            bp.reshape(BC, 3, 128, 4).transpose(2, 1, 0, 3))  # [128, 3, BC, 4]
        confc = np.ascontiguousarray(
            conf_pad[sl].reshape(BC, 3, 128).transpose(2, 1, 0))  # [128, 3, BC]
        in_maps.append({
            "seg2": np.ascontiguousarray(seg16[sl]),
            "boxc": boxc,
            "confc": confc,
            "boxr": np.ascontiguousarray(bp),
        })
    return in_maps


def combine_outputs(results):
    parts = np.array([results[i]["out"][0, 0] for i in range(NCORES)],
                     dtype=np.float32)
    return np.array(np.sum(parts, dtype=np.float32) / np.float32(B * N),
                    dtype=np.float32)


def kernel(det_boxes, det_confidence, seg_masks):
    nc = _get_nc()
    in_maps = make_in_maps(det_boxes, det_confidence, seg_masks)
    res = run_bass_kernel_spmd(nc, in_maps, list(range(NCORES)))
    return combine_outputs(res.results)
